# revision 1
# baseline (speedup 1.0000x reference)
"""AffinityLoss kernel for 8 Trainium2 NeuronCores (Bass/Tile, SPMD).

loss = mean over levels of mean(|softmax_b(G1) - softmax_b(G2)|), where
G[b] = r[b].T @ r[b] is the per-batch Gram matrix over hw pixels and the
softmax runs over the batch axis (b=4).

Strategy
--------
- Shard Gram ROWS across the 8 cores. Since |A1-A2| is symmetric in (i,j),
  only the upper triangle is computed (level 0); row-blocks are assigned to
  cores in a balanced mix {k, 15-k, 16+k, 31-k} so every core gets exactly
  34 level-0 tile positions. Level 1 is small and processed flat.
- One uniform SPMD program; all per-core variation (which rows / which
  j-columns / diagonal masks) is baked into host-prepared input data, so a
  single compiled NEFF runs on all 8 cores.
- Per position (128-row block x 256-col j-tile, both feature sets at once):
    PE:   8 matmuls (fp32) -> PSUM [128, 2, 4, 256] (side, batch, j)
    DVE:  m = max_b G (strided reduce); gsub = G - m  (softmax is invariant
          to per-(i,j) shifts; after this, exp args <= 0 and s in [1,4],
          so there is no overflow/underflow for any input data)
    ACT:  e = exp(gsub) -> bf16
    DVE:  s = sum_b e; rinv = 1/s; A = e*rinv; d = A1 - A2  (bf16, 2x mode)
    ACT:  |d| with accum_out -> per-position per-partition partial sums
- Host gathers the [128, 38] partial-sum tiles from the 8 cores and reduces
  in float64. Diagonal-straddling tiles carry {1, 0.5, 0} masks so that
  2*sum(upper) reproduces the full-matrix sum exactly.
"""

import numpy as np
import ml_dtypes

import concourse.bass as bass
import concourse.mybir as mybir
import concourse.tile as tile
from concourse.bass_utils import run_bass_kernel_spmd

F32 = mybir.dt.float32
BF16 = mybir.dt.bfloat16

B = 4
C0, HW0 = 64, 4096     # level 0: [4, 64, 64, 64]
C1, HW1 = 128, 1024    # level 1: [4, 128, 32, 32]
NCORES = 8
JT = 256               # j-tile width
RB = 128               # row-block height (partition dim)

N_L1 = HW1 // JT                    # 4
N_L0_TRI = 34                       # balanced upper-triangle positions/core
NPOS = N_L0_TRI + N_L1              # 38

TRI_BLOCKS = [sorted({k, 15 - k, 16 + k, 31 - k}) for k in range(NCORES)]


def _tri_positions_for_core(k):
    """(block, jt) list for core k: the 4 diagonal positions first."""
    blocks = TRI_BLOCKS[k]
    diag = [(b, b // 2) for b in blocks]
    off = []
    for b in blocks:
        for jt in range(b // 2 + 1, HW0 // JT):
            off.append((b, jt))
    return diag + off


def _bcast_ap(ap, dim_idx, n):
    """Insert a [0, n] broadcast free-dim at free position dim_idx."""
    new = [list(d) for d in ap.ap]
    new.insert(dim_idx + 1, [0, n])
    return bass.AP(tensor=ap.tensor, offset=ap.offset, ap=new)


def _split_excess_waits(nc, max_waits=1):
    """This walrus build accepts at most one sync-wait per instruction;
    spill extra waits onto preceding same-engine nops."""
    for f in nc.m.functions:
        for b in f.blocks:
            i = 0
            insts = b.instructions
            while i < len(insts):
                inst = insts[i]
                si = inst.sync_info
                if si is not None and si.on_wait and len(si.on_wait) > max_waits:
                    waits = list(si.on_wait)
                    keep = waits[-max_waits:]
                    spill = waits[:-max_waits]
                    si.on_wait = keep
                    inst.sync_info = si
                    new_nops = []
                    for j in range(0, len(spill), max_waits):
                        chunk = spill[j:j + max_waits]
                        nop = nc.engines[inst.engine].nop(nofuse=True).ins
                        nop.sync_info = mybir.SyncInfo(on_wait=chunk, on_update=[])
                        new_nops.append(nop)
                    for f2 in nc.m.functions:
                        for b2 in f2.blocks:
                            for nop in new_nops:
                                if nop in b2.instructions:
                                    b2.instructions.remove(nop)
                    for j, nop in enumerate(new_nops):
                        insts.insert(i + j, nop)
                    i += len(new_nops)
                i += 1


def _build_program():
    nc = bass.Bass()

    def param(name, shape, dt_):
        return nc.declare_dram_parameter(name, shape, dt_, isOutput=False)

    # gathered per-position operand chunks (channel-major: contiguous DMA)
    l0 = [param(f"lg0_{s}", [N_L0_TRI, C0, B, RB], F32) for s in (0, 1)]
    r0 = [param(f"rg0_{s}", [N_L0_TRI, C0, B, JT], F32) for s in (0, 1)]
    l1 = [param(f"l1_{s}", [C1, B, RB], F32) for s in (0, 1)]
    r1 = [param(f"r1_{s}", [C1, B, HW1], F32) for s in (0, 1)]
    masks_in = param("masks", [4, RB, JT], BF16)
    acc_out = nc.declare_dram_parameter("acc", [RB, NPOS], F32, isOutput=True)

    positions = [(0, p, p, p if p < 4 else None) for p in range(N_L0_TRI)]
    positions += [(1, 0, q, None) for q in range(N_L1)]

    with tile.TileContext(nc) as tc:
        with (
            tc.tile_pool(name="resident", bufs=1) as res_pool,
            tc.tile_pool(name="chunks", bufs=3) as chunk_pool,
            tc.tile_pool(name="work", bufs=3) as work_pool,
            tc.tile_pool(name="psum", bufs=2, space="PSUM") as psum_pool,
        ):
            masks = res_pool.tile([RB, 4, JT], BF16, tag="masks")
            nc.sync.dma_start(out=masks, in_=masks_in[:, :, :].rearrange("m p j -> p m j"))

            l1_t, r1_t = [], []
            for s in (0, 1):
                t = res_pool.tile([C1, B, RB], F32, tag=f"l1_{s}")
                nc.sync.dma_start(out=t, in_=l1[s][:, :, :])
                l1_t.append(t)
                t = res_pool.tile([C1, B, HW1], F32, tag=f"r1_{s}")
                nc.sync.dma_start(out=t, in_=r1[s][:, :, :])
                r1_t.append(t)

            acc = res_pool.tile([RB, NPOS], F32, tag="acc")

            for p, (lvl, lsel, rsel, mask_slot) in enumerate(positions):
                if lvl == 0:
                    rch, lch = [], []
                    for s in (0, 1):
                        t = chunk_pool.tile([C0, B, JT], F32, tag=f"rch{s}")
                        nc.sync.dma_start(out=t, in_=r0[s][rsel, :, :, :])
                        rch.append(t)
                        tl = chunk_pool.tile([C0, B, RB], F32, tag=f"lch{s}")
                        nc.sync.dma_start(out=tl, in_=l0[s][lsel, :, :, :])
                        lch.append(tl)

                ps = psum_pool.tile([RB, 2, B, JT], F32, tag="gram")
                for s in (0, 1):
                    for b in range(B):
                        if lvl == 0:
                            lhsT = lch[s][:, b, :]
                            rhs = rch[s][:, b, :]
                        else:
                            lhsT = l1_t[s][:, b, :]
                            rhs = r1_t[s][:, b, rsel * JT:(rsel + 1) * JT]
                        nc.tensor.matmul(ps[:, s, b, :], lhsT, rhs, start=True,
                                         stop=True)

                # m = max_b gram (single strided reduce; PSUM has 1 DVE read port)
                mf = work_pool.tile([RB, 2, JT], F32, tag="mf")
                nc.vector.tensor_reduce(
                    out=mf, in_=ps.rearrange("p s b j -> p s j b"),
                    axis=mybir.AxisListType.X, op=mybir.AluOpType.max,
                )
                gsub = work_pool.tile([RB, 2, B, JT], F32, tag="gsub")
                nc.vector.tensor_sub(gsub, ps, _bcast_ap(mf, 1, B))

                e = work_pool.tile([RB, 2, B, JT], BF16, tag="e")
                nc.scalar.activation(out=e, in_=gsub, func=mybir.ActivationFunctionType.Exp)

                spair = work_pool.tile([RB, 2, 2, JT], BF16, tag="spair")
                nc.gpsimd.tensor_add(spair, e[:, :, 0:2, :], e[:, :, 2:4, :])
                ssum = work_pool.tile([RB, 2, JT], BF16, tag="ssum")
                nc.gpsimd.tensor_add(ssum, spair[:, :, 0, :], spair[:, :, 1, :])
                rf = work_pool.tile([RB, 2, JT], F32, tag="rinv_f")
                nc.vector.reciprocal(out=rf, in_=ssum)
                rinvb = work_pool.tile([RB, 2, JT], BF16, tag="rinv_b")
                nc.gpsimd.tensor_copy(rinvb, rf)

                a_t = work_pool.tile([RB, 2, B, JT], BF16, tag="a")
                nc.gpsimd.tensor_mul(a_t, e, _bcast_ap(rinvb, 1, B))
                d = work_pool.tile([RB, B, JT], BF16, tag="d")
                nc.gpsimd.tensor_sub(d, a_t[:, 0, :, :], a_t[:, 1, :, :])

                if mask_slot is not None:
                    nc.vector.tensor_mul(d, d, _bcast_ap(masks[:, mask_slot, :], 0, B))

                scratch = work_pool.tile([RB, B, JT], BF16, tag="scratch")
                nc.scalar.activation(
                    out=scratch, in_=d, func=mybir.ActivationFunctionType.Abs,
                    accum_out=acc[:, p:p + 1],
                )

            nc.sync.dma_start(out=acc_out[:, :], in_=acc)

    _split_excess_waits(nc, 1)
    return nc


def _make_in_maps(fea1_0, fea1_1, fea2_0, fea2_1):
    # channel-major (c, b, hw) so every DMA line is contiguous
    r0v = [np.ascontiguousarray(np.asarray(fea1_0, dtype=np.float32).reshape(B, C0, HW0).transpose(1, 0, 2)),
           np.ascontiguousarray(np.asarray(fea2_0, dtype=np.float32).reshape(B, C0, HW0).transpose(1, 0, 2))]
    r1v = [np.ascontiguousarray(np.asarray(fea1_1, dtype=np.float32).reshape(B, C1, HW1).transpose(1, 0, 2)),
           np.ascontiguousarray(np.asarray(fea2_1, dtype=np.float32).reshape(B, C1, HW1).transpose(1, 0, 2))]

    in_maps = []
    for k in range(NCORES):
        m = {}
        row1 = slice(RB * k, RB * (k + 1))
        for s in (0, 1):
            m[f"l1_{s}"] = np.ascontiguousarray(r1v[s][:, :, row1])
            m[f"r1_{s}"] = r1v[s]
        pos = _tri_positions_for_core(k)
        for s in (0, 1):
            lg = np.empty((N_L0_TRI, C0, B, RB), np.float32)
            rg = np.empty((N_L0_TRI, C0, B, JT), np.float32)
            for p, (bl, jt) in enumerate(pos):
                lg[p] = r0v[s][:, :, bl * RB:(bl + 1) * RB]
                rg[p] = r0v[s][:, :, jt * JT:(jt + 1) * JT]
            m[f"lg0_{s}"] = lg
            m[f"rg0_{s}"] = rg
        mask = np.zeros((4, RB, JT), ml_dtypes.bfloat16)
        for slot, (bl, jt) in enumerate(pos[:4]):
            i = np.arange(RB)[:, None]
            j = np.arange(JT)[None, :]
            gi = bl * RB + i
            gj = jt * JT + j
            w = np.where(gj > gi, 1.0, np.where(gj == gi, 0.5, 0.0))
            mask[slot] = w.astype(ml_dtypes.bfloat16)
        m["masks"] = mask
        in_maps.append(m)
    return in_maps


_NC_CACHE = None


def kernel(fea1_0, fea1_1, fea2_0, fea2_1):
    global _NC_CACHE
    if _NC_CACHE is None:
        _NC_CACHE = _build_program()
    nc = _NC_CACHE
    in_maps = _make_in_maps(fea1_0, fea1_1, fea2_0, fea2_1)
    res = run_bass_kernel_spmd(nc, in_maps, core_ids=list(range(NCORES)))
    s0 = 0.0
    s1 = 0.0
    for r in res.results:
        acc = r["acc"].astype(np.float64)
        s0 += acc[:, :N_L0_TRI].sum()
        s1 += acc[:, N_L0_TRI:].sum()
    s0 *= 2.0  # upper triangle with {1, .5, 0} diag masks -> full-matrix sum
    loss = 0.5 * (s0 / (B * HW0 * HW0) + s1 / (B * HW1 * HW1))
    return np.float32(loss)



# revision 2
# speedup vs baseline: 2.0198x; 2.0198x over previous
"""AffinityLoss kernel for 8 Trainium2 NeuronCores (Bass/Tile, SPMD).

loss = mean over levels of mean(|softmax_b(G1) - softmax_b(G2)|), where
G[b] = r[b].T @ r[b] per-batch Gram over hw pixels, softmax over b (B=4).

Strategy (v2)
-------------
- Gram rows sharded across 8 cores; upper triangle only for level 0
  (balanced {k, 15-k, 16+k, 31-k} row-block mix, 34 tile positions/core),
  level 1 flat (4 positions/core). One SPMD program; per-core variation
  baked into host-gathered chunk data.
- Softmax-over-batch via a PIVOT batch b* (host-picked per j-tile, the
  batch with the largest column norm): softmax(G_b) is invariant to
  subtracting G_{b*}, and the subtraction is fused into the PE via
  contraction-concat  [f_b; -f_b*]^T [f_b; +f_b*] = G_b - G_b*  (one
  128-contraction bf16 matmul per (side, slot) for level 0).
- ACT computes e'_b = exp(G_b - G_b* - 60) for the 3 non-pivot slots in
  one instruction (bias -60 keeps everything in range; the pivot term is
  analytically e^-60). DVE clamps e' at 1e38 (washes any exp overflow
  into finite values, making sim and HW agree), computes rinv = s'^-1
  via tensor_scalar(pow) at 4x rate, and does the fused |.|-reduce via
  tensor_scalar(abs_max, accum_out). Pool does the sums, one side of the
  A-multiply, and the pivot-slot diff.
- acc_b[:, p] = sum_j |A1_b - A2_b| (non-pivot slots);
  acc_0[:, p] = sum_j |rinv1 - rinv2| (pivot slot, host scales by e^-60).
"""

import numpy as np
import ml_dtypes

import concourse.bass as bass
import concourse.mybir as mybir
import concourse.tile as tile
from concourse.bass_utils import run_bass_kernel_spmd

F32 = mybir.dt.float32
BF16 = mybir.dt.bfloat16
ALU = mybir.AluOpType

B = 4
C0, HW0 = 64, 4096     # level 0: [4, 64, 64, 64]
C1, HW1 = 128, 1024    # level 1: [4, 128, 32, 32]
NCORES = 8
JT = 256               # j-tile width
RB = 128               # row-block height
N_L0 = 34              # balanced upper-triangle positions/core
N_L1 = HW1 // JT       # 4
NPOS = N_L0 + N_L1     # 38
EBIAS = 60.0
EXC = float(np.exp(-EBIAS))          # e^-60, pivot-slot numerator
CLAMP = 1.0e38

TRI_BLOCKS = [sorted({k, 15 - k, 16 + k, 31 - k}) for k in range(NCORES)]


def _tri_positions_for_core(k):
    """(block, jt) list for core k: the 4 diagonal positions first."""
    blocks = TRI_BLOCKS[k]
    diag = [(b, b // 2) for b in blocks]
    off = []
    for b in blocks:
        for jt in range(b // 2 + 1, HW0 // JT):
            off.append((b, jt))
    return diag + off


def _bcast_ap(ap, dim_idx, n):
    """Insert a [0, n] broadcast free-dim at free position dim_idx."""
    new = [list(d) for d in ap.ap]
    new.insert(dim_idx + 1, [0, n])
    return bass.AP(tensor=ap.tensor, offset=ap.offset, ap=new)


def _split_excess_waits(nc, max_waits=1):
    """This walrus build accepts at most one sync-wait per instruction;
    spill extra waits onto preceding same-engine nops."""
    for f in nc.m.functions:
        for b in f.blocks:
            i = 0
            insts = b.instructions
            while i < len(insts):
                inst = insts[i]
                si = inst.sync_info
                if si is not None and si.on_wait and len(si.on_wait) > max_waits:
                    waits = list(si.on_wait)
                    keep = waits[-max_waits:]
                    spill = waits[:-max_waits]
                    si.on_wait = keep
                    inst.sync_info = si
                    new_nops = []
                    for j in range(0, len(spill), max_waits):
                        chunk = spill[j:j + max_waits]
                        nop = nc.engines[inst.engine].nop(nofuse=True).ins
                        nop.sync_info = mybir.SyncInfo(on_wait=chunk, on_update=[])
                        new_nops.append(nop)
                    for f2 in nc.m.functions:
                        for b2 in f2.blocks:
                            for nop in new_nops:
                                if nop in b2.instructions:
                                    b2.instructions.remove(nop)
                    for j, nop in enumerate(new_nops):
                        insts.insert(i + j, nop)
                    i += len(new_nops)
                i += 1


def _build_program():
    nc = bass.Bass()

    def param(name, shape, dt_):
        return nc.declare_dram_parameter(name, shape, dt_, isOutput=False)

    # merged concat chunks: [pos, 128, 2, 3, 384]
    #   rows 0:C0 slot data, rows C0: pivot (lhsT cols 0:128 | rhs cols 128:384)
    ch0 = param("ch0", [N_L0, 2 * C0, 2, 3, RB + JT], BF16)
    # level 1: [pos, 128, 2, 4, 384], slot 0 = pivot (-f|+f), slots 1..3 data
    ch1 = param("ch1", [N_L1, C1, 2, 4, RB + JT], BF16)
    masks_in = param("masks", [4, RB, JT], BF16)
    acc_b_out = nc.declare_dram_parameter("acc_b", [RB, NPOS], F32, isOutput=True)
    acc_0_out = nc.declare_dram_parameter("acc_0", [RB, NPOS], F32, isOutput=True)

    lp = nc.allow_low_precision(reason="bf16 softmax pipeline, fp64 host reduce")
    lp.__enter__()

    with tile.TileContext(nc) as tc:
        with (
            tc.tile_pool(name="res", bufs=1) as res_pool,
            tc.tile_pool(name="chunks", bufs=3) as chunk_pool,
            tc.tile_pool(name="work", bufs=2) as work_pool,
            tc.tile_pool(name="small", bufs=3) as small_pool,
            tc.tile_pool(name="psum", bufs=2, space="PSUM") as psum_pool,
        ):
            masks = res_pool.tile([RB, 4, JT], BF16, tag="masks")
            nc.sync.dma_start(out=masks, in_=masks_in[:, :, :].rearrange("m p j -> p m j"))
            biast = res_pool.tile([RB, 1], F32, tag="biast")
            nc.vector.memset(biast, -EBIAS)
            acc_b = res_pool.tile([RB, NPOS], F32, tag="acc_b")
            acc_0 = res_pool.tile([RB, NPOS], F32, tag="acc_0")

            for p in range(NPOS):
                lvl = 0 if p < N_L0 else 1
                dma_eng = nc.sync if p % 2 == 0 else nc.scalar
                if lvl == 0:
                    ch = chunk_pool.tile([2 * C0, 2, 3, RB + JT], BF16, tag="ch0")
                    dma_eng.dma_start(out=ch, in_=ch0[p, :, :, :, :])
                else:
                    ch = chunk_pool.tile([C1, 2, 4, RB + JT], BF16, tag="ch1")
                    dma_eng.dma_start(out=ch, in_=ch1[p - N_L0, :, :, :, :])

                ps = psum_pool.tile([RB, 2, 3, JT], F32, tag="gram")
                for s in (0, 1):
                    for m in range(3):
                        if lvl == 0:
                            nc.tensor.matmul(ps[:, s, m, :],
                                             ch[:, s, m, 0:RB],
                                             ch[:, s, m, RB:RB + JT],
                                             start=True, stop=True)
                        else:
                            nc.tensor.matmul(ps[:, s, m, :],
                                             ch[:, s, m + 1, 0:RB],
                                             ch[:, s, m + 1, RB:RB + JT],
                                             start=True, stop=False)
                            nc.tensor.matmul(ps[:, s, m, :],
                                             ch[:, s, 0, 0:RB],
                                             ch[:, s, 0, RB:RB + JT],
                                             start=False, stop=True)

                # e' = exp(G_b - G_piv - 60) -> bf16
                e = work_pool.tile([RB, 2, 3, JT], BF16, tag="e")
                nc.scalar.activation(out=e, in_=ps,
                                     func=mybir.ActivationFunctionType.Exp,
                                     bias=biast)
                # clamp washes +-inf from exp overflow (rare pivot misses)
                ec = work_pool.tile([RB, 2, 3, JT], BF16, tag="ec")
                nc.vector.tensor_scalar(out=ec, in0=e, scalar1=CLAMP,
                                        scalar2=None, op0=ALU.min)

                # s' = e^-60 + ec0 + ec1 + ec2   (Pool)
                p1 = small_pool.tile([RB, 2, JT], BF16, tag="p1")
                nc.gpsimd.tensor_add(p1, ec[:, :, 0, :], ec[:, :, 1, :])
                sden = small_pool.tile([RB, 2, JT], BF16, tag="sden")
                nc.gpsimd.scalar_tensor_tensor(out=sden, in0=ec[:, :, 2, :],
                                               scalar=EXC, in1=p1,
                                               op0=ALU.add, op1=ALU.add)
                # rinv = s'^-1 (DVE tensor_scalar pow, 4x)
                rinv = small_pool.tile([RB, 2, JT], BF16, tag="rinv")
                nc.vector.tensor_scalar(out=rinv, in0=sden, scalar1=-1.0,
                                        scalar2=None, op0=ALU.pow)

                # A = ec * rinv ; side 0 on DVE, side 1 on Pool
                a_t = work_pool.tile([RB, 2, 3, JT], BF16, tag="a")
                nc.vector.tensor_mul(a_t[:, 0], ec[:, 0], _bcast_ap(rinv[:, 0, :], 0, 3))
                nc.gpsimd.tensor_mul(a_t[:, 1], ec[:, 1], _bcast_ap(rinv[:, 1, :], 0, 3))

                # d_b = A1 - A2 (DVE), d0 = rinv1 - rinv2 (Pool)
                d = work_pool.tile([RB, 3, JT], BF16, tag="d")
                nc.vector.tensor_sub(d, a_t[:, 0], a_t[:, 1])
                d0 = small_pool.tile([RB, JT], BF16, tag="d0")
                nc.gpsimd.tensor_sub(d0, rinv[:, 0, :], rinv[:, 1, :])

                if lvl == 0 and p < 4:  # diagonal positions carry tri masks
                    nc.gpsimd.tensor_mul(d, d, _bcast_ap(masks[:, p, :], 0, 3))
                    nc.gpsimd.tensor_mul(d0, d0, masks[:, p, :])

                # fused |.| + accumulate along free dim (DVE tensor_scalar 4x)
                scr = work_pool.tile([RB, 3, JT], BF16, tag="scr")
                nc.vector.tensor_scalar(out=scr, in0=d, scalar1=0.0, scalar2=None,
                                        op0=ALU.abs_max, op1=ALU.add,
                                        accum_out=acc_b[:, p:p + 1])
                scr0 = small_pool.tile([RB, JT], BF16, tag="scr0")
                nc.vector.tensor_scalar(out=scr0, in0=d0, scalar1=0.0, scalar2=None,
                                        op0=ALU.abs_max, op1=ALU.add,
                                        accum_out=acc_0[:, p:p + 1])

            nc.sync.dma_start(out=acc_b_out[:, :], in_=acc_b)
            nc.sync.dma_start(out=acc_0_out[:, :], in_=acc_0)

    _split_excess_waits(nc, 1)
    return nc


def _make_in_maps(fea1_0, fea1_1, fea2_0, fea2_1):
    bf = ml_dtypes.bfloat16
    # channel-major (c, b, hw), contiguous DMA lines
    r0 = [np.ascontiguousarray(np.asarray(f, dtype=np.float32).reshape(B, C0, HW0)
                               .transpose(1, 0, 2)).astype(bf)
          for f in (fea1_0, fea2_0)]
    r1 = [np.ascontiguousarray(np.asarray(f, dtype=np.float32).reshape(B, C1, HW1)
                               .transpose(1, 0, 2)).astype(bf)
          for f in (fea1_1, fea2_1)]

    # per-j-tile pivot batch: largest column norm over both sides
    def pivots(rs, hw, c):
        n2 = np.zeros((B, hw), np.float32)
        for r in rs:
            n2 = np.maximum(n2, (r.astype(np.float32) ** 2).sum(axis=0))
        piv = []
        for jt in range(hw // JT):
            piv.append(int(n2[:, jt * JT:(jt + 1) * JT].max(axis=1).argmax()))
        return piv

    piv0 = pivots(r0, HW0, C0)
    piv1 = pivots(r1, HW1, C1)

    in_maps = []
    for k in range(NCORES):
        pos = _tri_positions_for_core(k)
        ch0 = np.empty((N_L0, 2 * C0, 2, 3, RB + JT), bf)
        for p, (bl, jt) in enumerate(pos):
            bstar = piv0[jt]
            slots = [b for b in range(B) if b != bstar]
            rsl = slice(bl * RB, (bl + 1) * RB)
            jsl = slice(jt * JT, (jt + 1) * JT)
            for s in (0, 1):
                for m, b in enumerate(slots):
                    ch0[p, 0:C0, s, m, 0:RB] = r0[s][:, b, rsl]
                    ch0[p, 0:C0, s, m, RB:] = r0[s][:, b, jsl]
                    ch0[p, C0:, s, m, 0:RB] = -r0[s][:, bstar, rsl]
                    ch0[p, C0:, s, m, RB:] = r0[s][:, bstar, jsl]

        ch1 = np.empty((N_L1, C1, 2, 4, RB + JT), bf)
        rsl = slice(k * RB, (k + 1) * RB)
        for q in range(N_L1):
            bstar = piv1[q]
            slots = [b for b in range(B) if b != bstar]
            jsl = slice(q * JT, (q + 1) * JT)
            for s in (0, 1):
                ch1[q, :, s, 0, 0:RB] = -r1[s][:, bstar, rsl]
                ch1[q, :, s, 0, RB:] = r1[s][:, bstar, jsl]
                for m, b in enumerate(slots):
                    ch1[q, :, s, m + 1, 0:RB] = r1[s][:, b, rsl]
                    ch1[q, :, s, m + 1, RB:] = r1[s][:, b, jsl]

        mask = np.zeros((4, RB, JT), bf)
        for slot, (bl, jt) in enumerate(pos[:4]):
            gi = bl * RB + np.arange(RB)[:, None]
            gj = jt * JT + np.arange(JT)[None, :]
            mask[slot] = np.where(gj > gi, 1.0, np.where(gj == gi, 0.5, 0.0)).astype(bf)

        in_maps.append({"ch0": ch0, "ch1": ch1, "masks": mask})
    return in_maps


_NC_CACHE = None


def kernel(fea1_0, fea1_1, fea2_0, fea2_1):
    global _NC_CACHE
    if _NC_CACHE is None:
        _NC_CACHE = _build_program()
    nc = _NC_CACHE
    in_maps = _make_in_maps(fea1_0, fea1_1, fea2_0, fea2_1)
    res = run_bass_kernel_spmd(nc, in_maps, core_ids=list(range(NCORES)))
    s0 = 0.0
    s1 = 0.0
    exc = float(np.exp(np.float64(-EBIAS)))
    for r in res.results:
        ab = r["acc_b"].astype(np.float64)
        a0 = r["acc_0"].astype(np.float64)
        s0 += ab[:, :N_L0].sum() + exc * a0[:, :N_L0].sum()
        s1 += ab[:, N_L0:].sum() + exc * a0[:, N_L0:].sum()
    s0 *= 2.0  # upper triangle with {1, .5, 0} diag masks -> full-matrix sum
    loss = 0.5 * (s0 / (B * HW0 * HW0) + s1 / (B * HW1 * HW1))
    return np.float32(loss)
